# revision 13
# baseline (speedup 1.0000x reference)
"""DirectionalLoss Trainium2 kernel.

Computes total = 0.5*MSE + 0.5*(directional_loss + correlation_loss)/2 for
predictions/targets of shape [8192, 4096] f32, data-parallel over 8 cores
(1024 rows per core).

Per-core, per row-tile [128, 4096]:
  ACT   : Square(x)+accum -> Sxx, Square(y)+accum -> Syy,
          Copy(x)+accum -> Sx,  Copy(y)+accum -> Sy
  DVE   : tensor_tensor_reduce(x*y)+accum -> Sxy,
          pc = x[:,1:]-x[:,:-1], prod = pc*tc,
          tensor_scalar(is_gt 0)+accum -> cnt_pos
  GPSIMD: tc = y[:,1:]-y[:,:-1], tensor_scalar(is_eq 0)+accum -> cnt_zero

Per-row math (on-chip epilogue over [128, 8] stats):
  corr = (Sxy/H - mx*my) / ((sx+eps)*(sy+eps)),  sx = sqrt((Sxx - Sx^2/H)/(H-1))
  MSE partial = Sxx + Syy - 2*Sxy
  match count = cnt_pos + cnt_zero        (match <=> pc*tc>0 or tc==0)
Cross-partition reduction via ones-matmul on the TensorEngine; each core
outputs 3 scalars which the host combines.
"""

import sys

for _p in ("/opt/trn_rl_repo", "/root/.axon_site/_ro/trn_rl_repo"):
    if _p not in sys.path:
        sys.path.insert(0, _p)

import numpy as np

import concourse.bass as bass
import concourse.tile as tile
from concourse import mybir
from concourse.bass_utils import run_bass_kernel_spmd

B_FULL = 8192
H = 4096
N_CORES = 8
ROWS_PER_CORE = B_FULL // N_CORES  # 1024
P = 128
N_TILES = ROWS_PER_CORE // P  # 8
EPSILON = 1e-6
MSE_WEIGHT = 0.5
DIRECTIONAL_WEIGHT = 0.5

F32 = mybir.dt.float32
Alu = mybir.AluOpType
Act = mybir.ActivationFunctionType


def _split_multiwait(nc, limit=1):
    """Hoist semaphore waits beyond `limit` into single-wait NoOps placed
    just before the owning instruction (same engine, so program order
    preserves the wait point). The walrus build in this container rejects
    instructions whose encoding has no room for >1 sync wait (e.g. the
    kernel-tail reset drain collects one wait per live semaphore)."""
    k = 0
    for f in nc.m.functions:
        for bb in f.blocks:
            insts = list(bb.instructions)
            out = []
            for ins in insts:
                si = ins.sync_info
                waits = list(si.on_wait) if si is not None and si.on_wait else []
                if len(waits) > limit:
                    spill, keep = waits[:-limit], waits[-limit:]
                    for w in spill:
                        k += 1
                        out.append(
                            mybir.InstNoOp(
                                name=f"waitnop-{k}",
                                engine=ins.engine,
                                sync_info=mybir.SyncInfo(on_wait=[w], on_update=[]),
                            )
                        )
                    ins.sync_info = mybir.SyncInfo(
                        on_wait=keep, on_update=list(si.on_update or [])
                    )
                out.append(ins)
            if len(out) != len(insts):
                bb.instructions = out


def build_bass(split_waits=True):
    nc = bass.Bass()
    x_d = nc.dram_tensor("x", [ROWS_PER_CORE, H], F32, kind="ExternalInput")
    y_d = nc.dram_tensor("y", [ROWS_PER_CORE, H], F32, kind="ExternalInput")
    out_d = nc.dram_tensor("out", [1, 3], F32, kind="ExternalOutput")

    with tile.TileContext(nc) as tc:
        with (
            tc.tile_pool(name="xin", bufs=2) as xin,
            tc.tile_pool(name="yin", bufs=2) as yin,
            tc.tile_pool(name="pcp", bufs=1) as pcp,
            tc.tile_pool(name="tcp", bufs=2) as tcp,
            tc.tile_pool(name="prodp", bufs=1) as prodp,
            tc.tile_pool(name="deadp", bufs=1) as deadp,
            tc.tile_pool(name="stats", bufs=1) as stats,
            tc.tile_pool(name="psum", bufs=1, space="PSUM") as psum_pool,
        ):
            sx = stats.tile([P, N_TILES], F32)
            sy = stats.tile([P, N_TILES], F32)
            sxx = stats.tile([P, N_TILES], F32)
            syy = stats.tile([P, N_TILES], F32)
            sxy = stats.tile([P, N_TILES], F32)
            cnt1 = stats.tile([P, N_TILES], F32)
            cnt2 = stats.tile([P, N_TILES], F32)
            ones = stats.tile([P, 1], F32)
            nc.vector.memset(ones[:], 1.0)

            for i in range(N_TILES):
                xt = xin.tile([P, H], F32)
                yt = yin.tile([P, H], F32)
                nc.sync.dma_start(out=xt[:], in_=x_d[i * P : (i + 1) * P, :])
                nc.sync.dma_start(out=yt[:], in_=y_d[i * P : (i + 1) * P, :])

                prod = prodp.tile([P, H - 1], F32)
                pc = pcp.tile([P, H - 1], F32)
                tcd = tcp.tile([P, H - 1], F32)
                dead = deadp.tile([P, H], F32)

                # Sxy: out = (x+0)*y into a dead tile, accum_out = sum(x*y)
                nc.vector.scalar_tensor_tensor(
                    out=dead[:],
                    in0=xt[:],
                    scalar=0.0,
                    in1=yt[:],
                    op0=Alu.add,
                    op1=Alu.mult,
                    accum_out=sxy[:, i : i + 1],
                )

                # diffs
                nc.vector.tensor_tensor(
                    out=pc[:], in0=xt[:, 1:], in1=xt[:, : H - 1], op=Alu.subtract
                )
                nc.gpsimd.tensor_tensor(
                    out=tcd[:], in0=yt[:, 1:], in1=yt[:, : H - 1], op=Alu.subtract
                )

                # row sums of x, y, x^2, y^2 on the scalar engine. Each op
                # needs a full-width `out` it will never be read from; a
                # 0-stride AP over a private [P,1] tile keeps every ACT
                # instruction down to a single sync wait (its input DMA) —
                # the Activation encoding has no room for more.
                def act_dead(tag):
                    t = stats.tile([P, 1], F32, tag=tag)
                    return t.broadcast_to([P, H])

                nc.scalar.activation(
                    out=act_dead(f"dsx{i}"), in_=xt[:], func=Act.Copy,
                    accum_out=sx[:, i : i + 1],
                )
                nc.scalar.activation(
                    out=act_dead(f"dsy{i}"), in_=yt[:], func=Act.Copy,
                    accum_out=sy[:, i : i + 1],
                )
                nc.scalar.activation(
                    out=act_dead(f"dsxx{i}"), in_=xt[:], func=Act.Square,
                    accum_out=sxx[:, i : i + 1],
                )
                nc.scalar.activation(
                    out=act_dead(f"dsyy{i}"), in_=yt[:], func=Act.Square,
                    accum_out=syy[:, i : i + 1],
                )

                # prod = pc*tc, then count prod>0 (in-place), count tc==0 (in-place)
                nc.vector.tensor_tensor(
                    out=prod[:], in0=pc[:], in1=tcd[:], op=Alu.mult
                )
                nc.vector.tensor_scalar(
                    out=prod[:],
                    in0=prod[:],
                    scalar1=0.0,
                    scalar2=None,
                    op0=Alu.is_gt,
                    op1=Alu.add,
                    accum_out=cnt1[:, i : i + 1],
                )
                nc.vector.tensor_scalar(
                    out=tcd[:],
                    in0=tcd[:],
                    scalar1=0.0,
                    scalar2=None,
                    op0=Alu.is_equal,
                    op1=Alu.add,
                    accum_out=cnt2[:, i : i + 1],
                )

            # ---- epilogue: per-row corr + partial sums -> 3 scalars ----
            ep = stats
            sxsx = ep.tile([P, N_TILES], F32)
            sysy = ep.tile([P, N_TILES], F32)
            sxsy = ep.tile([P, N_TILES], F32)
            nc.vector.tensor_tensor(out=sxsx[:], in0=sx[:], in1=sx[:], op=Alu.mult)
            nc.vector.tensor_tensor(out=sysy[:], in0=sy[:], in1=sy[:], op=Alu.mult)
            nc.vector.tensor_tensor(out=sxsy[:], in0=sx[:], in1=sy[:], op=Alu.mult)

            ax = ep.tile([P, N_TILES], F32)
            ay = ep.tile([P, N_TILES], F32)
            nc.vector.scalar_tensor_tensor(
                out=ax[:], in0=sxsx[:], scalar=-1.0 / H, in1=sxx[:],
                op0=Alu.mult, op1=Alu.add,
            )
            nc.vector.scalar_tensor_tensor(
                out=ay[:], in0=sysy[:], scalar=-1.0 / H, in1=syy[:],
                op0=Alu.mult, op1=Alu.add,
            )
            sdx = ep.tile([P, N_TILES], F32)
            sdy = ep.tile([P, N_TILES], F32)
            nc.scalar.activation(
                out=sdx[:], in_=ax[:], func=Act.Sqrt, scale=1.0 / (H - 1)
            )
            nc.scalar.activation(
                out=sdy[:], in_=ay[:], func=Act.Sqrt, scale=1.0 / (H - 1)
            )
            nc.vector.tensor_scalar(
                out=sdx[:], in0=sdx[:], scalar1=EPSILON, scalar2=None, op0=Alu.add
            )
            nc.vector.tensor_scalar(
                out=sdy[:], in0=sdy[:], scalar1=EPSILON, scalar2=None, op0=Alu.add
            )
            den = ep.tile([P, N_TILES], F32)
            nc.vector.tensor_tensor(out=den[:], in0=sdx[:], in1=sdy[:], op=Alu.mult)
            rden = ep.tile([P, N_TILES], F32)
            nc.vector.reciprocal(out=rden[:], in_=den[:])

            num = ep.tile([P, N_TILES], F32)
            nc.vector.scalar_tensor_tensor(
                out=num[:], in0=sxsy[:], scalar=-1.0 / H, in1=sxy[:],
                op0=Alu.mult, op1=Alu.add,
            )
            corr = ep.tile([P, N_TILES], F32)
            nc.vector.scalar_tensor_tensor(
                out=corr[:], in0=num[:], scalar=1.0 / H, in1=rden[:],
                op0=Alu.mult, op1=Alu.mult,
            )

            stat3 = ep.tile([P, 4], F32)
            dead8 = ep.tile([P, N_TILES], F32)
            # col 0: sum of corr over the tile's rows
            nc.vector.tensor_scalar(
                out=dead8[:], in0=corr[:], scalar1=0.0, scalar2=None,
                op0=Alu.add, op1=Alu.add, accum_out=stat3[:, 0:1],
            )
            # col 1: sum over rows of (Sxx + Syy - 2*Sxy)
            t_m = ep.tile([P, N_TILES], F32)
            nc.vector.scalar_tensor_tensor(
                out=t_m[:], in0=sxy[:], scalar=-2.0, in1=sxx[:],
                op0=Alu.mult, op1=Alu.add,
            )
            dead8b = ep.tile([P, N_TILES], F32)
            nc.vector.scalar_tensor_tensor(
                out=dead8b[:], in0=t_m[:], scalar=0.0, in1=syy[:],
                op0=Alu.add, op1=Alu.add, accum_out=stat3[:, 1:2],
            )
            # col 2: total match count
            dead8c = ep.tile([P, N_TILES], F32)
            nc.vector.scalar_tensor_tensor(
                out=dead8c[:], in0=cnt1[:], scalar=0.0, in1=cnt2[:],
                op0=Alu.add, op1=Alu.add, accum_out=stat3[:, 2:3],
            )

            # cross-partition reduce: [1,3] = ones.T @ stat3[:, :3]
            acc = psum_pool.tile([1, 3], F32)
            nc.tensor.matmul(acc[:], ones[:], stat3[:, 0:3], start=True, stop=True)
            sb_out = ep.tile([1, 3], F32)
            nc.vector.tensor_copy(out=sb_out[:], in_=acc[:])
            nc.sync.dma_start(out=out_d[:], in_=sb_out[:])

    if split_waits:
        _split_multiwait(nc)
    return nc


_NC_CACHE = None


def _get_nc():
    global _NC_CACHE
    if _NC_CACHE is None:
        _NC_CACHE = build_bass()
    return _NC_CACHE


def run_cores(predictions, targets, **kwargs):
    """Run the SPMD kernel; returns (per-core [1,3] results list, BassKernelResults)."""
    nc = _get_nc()
    preds = np.ascontiguousarray(predictions, dtype=np.float32)
    targs = np.ascontiguousarray(targets, dtype=np.float32)
    in_maps = [
        {
            "x": preds[c * ROWS_PER_CORE : (c + 1) * ROWS_PER_CORE],
            "y": targs[c * ROWS_PER_CORE : (c + 1) * ROWS_PER_CORE],
        }
        for c in range(N_CORES)
    ]
    res = run_bass_kernel_spmd(nc, in_maps, core_ids=list(range(N_CORES)), **kwargs)
    return [r["out"] for r in res.results], res


def _combine(outs):
    corr_sum = 0.0
    mse_sum = 0.0
    cnt_sum = 0.0
    for o in outs:
        corr_sum += float(o[0, 0])
        mse_sum += float(o[0, 1])
        cnt_sum += float(o[0, 2])
    mse = mse_sum / (B_FULL * H)
    directional_loss = 1.0 - cnt_sum / (B_FULL * (H - 1))
    correlation_loss = (B_FULL - corr_sum) / (2.0 * B_FULL)
    dir_combined = (directional_loss + correlation_loss) / 2.0
    total = MSE_WEIGHT * mse + DIRECTIONAL_WEIGHT * dir_combined
    return np.float32(total)


def kernel(predictions, targets):
    outs, _ = run_cores(predictions, targets)
    return np.asarray(_combine(outs))


# revision 16
# speedup vs baseline: 1.0467x; 1.0467x over previous
"""DirectionalLoss Trainium2 kernel.

Computes total = 0.5*MSE + 0.5*(directional_loss + correlation_loss)/2 for
predictions/targets of shape [8192, 4096] f32, data-parallel over 8 cores
(1024 rows per core).

Per-core, per row-tile [128, 4096]:
  ACT   : Square(x)+accum -> Sxx, Square(y)+accum -> Syy,
          Copy(x)+accum -> Sx,  Copy(y)+accum -> Sy
  DVE   : tensor_tensor_reduce(x*y)+accum -> Sxy,
          pc = x[:,1:]-x[:,:-1], prod = pc*tc,
          tensor_scalar(is_gt 0)+accum -> cnt_pos
  GPSIMD: tc = y[:,1:]-y[:,:-1], tensor_scalar(is_eq 0)+accum -> cnt_zero

Per-row math (on-chip epilogue over [128, 8] stats):
  corr = (Sxy/H - mx*my) / ((sx+eps)*(sy+eps)),  sx = sqrt((Sxx - Sx^2/H)/(H-1))
  MSE partial = Sxx + Syy - 2*Sxy
  match count = cnt_pos + cnt_zero        (match <=> pc*tc>0 or tc==0)
Cross-partition reduction via ones-matmul on the TensorEngine; each core
outputs 3 scalars which the host combines.
"""

import sys

for _p in ("/opt/trn_rl_repo", "/root/.axon_site/_ro/trn_rl_repo"):
    if _p not in sys.path:
        sys.path.insert(0, _p)

import numpy as np

import concourse.bass as bass
import concourse.tile as tile
from concourse import mybir
from concourse.bass_utils import run_bass_kernel_spmd

B_FULL = 8192
H = 4096
N_CORES = 8
ROWS_PER_CORE = B_FULL // N_CORES  # 1024
P = 128
N_TILES = ROWS_PER_CORE // P  # 8
EPSILON = 1e-6
MSE_WEIGHT = 0.5
DIRECTIONAL_WEIGHT = 0.5

F32 = mybir.dt.float32
Alu = mybir.AluOpType
Act = mybir.ActivationFunctionType


def _split_multiwait(nc, limit=1):
    """Hoist semaphore waits beyond `limit` into single-wait NoOps placed
    just before the owning instruction (same engine, so program order
    preserves the wait point). The walrus build in this container rejects
    instructions whose encoding has no room for >1 sync wait (e.g. the
    kernel-tail reset drain collects one wait per live semaphore)."""
    k = 0
    for f in nc.m.functions:
        for bb in f.blocks:
            insts = list(bb.instructions)
            out = []
            for ins in insts:
                si = ins.sync_info
                waits = list(si.on_wait) if si is not None and si.on_wait else []
                if len(waits) > limit:
                    spill, keep = waits[:-limit], waits[-limit:]
                    for w in spill:
                        k += 1
                        out.append(
                            mybir.InstNoOp(
                                name=f"waitnop-{k}",
                                engine=ins.engine,
                                sync_info=mybir.SyncInfo(on_wait=[w], on_update=[]),
                            )
                        )
                    ins.sync_info = mybir.SyncInfo(
                        on_wait=keep, on_update=list(si.on_update or [])
                    )
                out.append(ins)
            if len(out) != len(insts):
                bb.instructions = out


def build_bass(split_waits=True):
    nc = bass.Bass()
    x_d = nc.dram_tensor("x", [ROWS_PER_CORE, H], F32, kind="ExternalInput")
    y_d = nc.dram_tensor("y", [ROWS_PER_CORE, H], F32, kind="ExternalInput")
    out_d = nc.dram_tensor("out", [1, 3], F32, kind="ExternalOutput")

    BF16 = mybir.dt.bfloat16
    with tile.TileContext(nc) as tc:
        with (
            tc.tile_pool(name="xin", bufs=2) as xin,
            tc.tile_pool(name="yin", bufs=2) as yin,
            tc.tile_pool(name="deadp", bufs=1) as deadp,
            tc.tile_pool(name="stats", bufs=1) as stats,
            tc.tile_pool(name="psum", bufs=1, space="PSUM") as psum_pool,
        ):
            sx = stats.tile([P, N_TILES], F32)
            sy = stats.tile([P, N_TILES], F32)
            sxx = stats.tile([P, N_TILES], F32)
            syy = stats.tile([P, N_TILES], F32)
            sxy = stats.tile([P, N_TILES], F32)
            cnt1 = stats.tile([P, N_TILES], F32)
            cnt2 = stats.tile([P, N_TILES], F32)
            ones = stats.tile([P, 1], F32)
            nc.vector.memset(ones[:], 1.0)

            # Diffs are stored as bf16: sign and exact-zero of
            # bf16(x1-x0) match the f32 computation (rounding preserves
            # sign; a zero requires x1 == x0 exactly), so the count math
            # stays bit-exact while the product runs in the DVE's 2x
            # mode and the two count passes in 4x mode. Tiles are padded
            # to an even width H with a -1 column so the counts can scan
            # the full (even) width: is_gt(-1,0)=0 and is_eq(-1,0)=0
            # contribute nothing. Count outputs go to a separate dead
            # tile so the pad columns survive across iterations.
            pc_b = stats.tile([P, H - 1], BF16)
            prod_b = stats.tile([P, H], BF16)
            tcd_a = stats.tile([P, H], BF16, tag="tcdA")
            tcd_b2 = stats.tile([P, H], BF16, tag="tcdB")
            tcd_bufs = [tcd_a, tcd_b2]
            mask_dead = stats.tile([P, H], BF16)
            nc.vector.memset(prod_b[:, H - 1 : H], -1.0)
            nc.vector.memset(tcd_bufs[0][:, H - 1 : H], -1.0)
            nc.vector.memset(tcd_bufs[1][:, H - 1 : H], -1.0)

            for i in range(N_TILES):
                xt = xin.tile([P, H], F32)
                yt = yin.tile([P, H], F32)
                nc.sync.dma_start(out=xt[:], in_=x_d[i * P : (i + 1) * P, :])
                nc.sync.dma_start(out=yt[:], in_=y_d[i * P : (i + 1) * P, :])

                pc = pc_b
                tcd = tcd_bufs[i % 2]
                dead = deadp.tile([P, H], F32)

                # Sxy: out = (x+0)*y into a dead tile, accum_out = sum(x*y)
                nc.vector.scalar_tensor_tensor(
                    out=dead[:],
                    in0=xt[:],
                    scalar=0.0,
                    in1=yt[:],
                    op0=Alu.add,
                    op1=Alu.mult,
                    accum_out=sxy[:, i : i + 1],
                )

                # diffs (f32 read, bf16 write)
                nc.vector.tensor_tensor(
                    out=pc[:], in0=xt[:, 1:], in1=xt[:, : H - 1], op=Alu.subtract
                )
                nc.gpsimd.tensor_tensor(
                    out=tcd[:, : H - 1], in0=yt[:, 1:], in1=yt[:, : H - 1],
                    op=Alu.subtract,
                )

                # row sums of x, y, x^2, y^2 on the scalar engine. Each op
                # needs a full-width `out` it will never be read from; a
                # 0-stride AP over a private [P,1] tile keeps every ACT
                # instruction down to a single sync wait (its input DMA) —
                # the Activation encoding has no room for more.
                def act_dead(tag):
                    t = stats.tile([P, 1], F32, tag=tag)
                    return t.broadcast_to([P, H])

                nc.scalar.activation(
                    out=act_dead(f"dsx{i}"), in_=xt[:], func=Act.Copy,
                    accum_out=sx[:, i : i + 1],
                )
                nc.scalar.activation(
                    out=act_dead(f"dsy{i}"), in_=yt[:], func=Act.Copy,
                    accum_out=sy[:, i : i + 1],
                )
                nc.scalar.activation(
                    out=act_dead(f"dsxx{i}"), in_=xt[:], func=Act.Square,
                    accum_out=sxx[:, i : i + 1],
                )
                nc.scalar.activation(
                    out=act_dead(f"dsyy{i}"), in_=yt[:], func=Act.Square,
                    accum_out=syy[:, i : i + 1],
                )

                # prod = pc*tc (bf16, 2x mode), then the two counts over the
                # padded even width (bf16 single-src, 4x mode)
                nc.vector.tensor_tensor(
                    out=prod_b[:, : H - 1], in0=pc[:], in1=tcd[:, : H - 1],
                    op=Alu.mult,
                )
                nc.vector.tensor_scalar(
                    out=mask_dead[:],
                    in0=prod_b[:],
                    scalar1=0.0,
                    scalar2=None,
                    op0=Alu.is_gt,
                    op1=Alu.add,
                    accum_out=cnt1[:, i : i + 1],
                )
                nc.vector.tensor_scalar(
                    out=mask_dead[:],
                    in0=tcd[:],
                    scalar1=0.0,
                    scalar2=None,
                    op0=Alu.is_equal,
                    op1=Alu.add,
                    accum_out=cnt2[:, i : i + 1],
                )

            # ---- epilogue: per-row corr + partial sums -> 3 scalars ----
            ep = stats
            sxsx = ep.tile([P, N_TILES], F32)
            sysy = ep.tile([P, N_TILES], F32)
            sxsy = ep.tile([P, N_TILES], F32)
            nc.vector.tensor_tensor(out=sxsx[:], in0=sx[:], in1=sx[:], op=Alu.mult)
            nc.vector.tensor_tensor(out=sysy[:], in0=sy[:], in1=sy[:], op=Alu.mult)
            nc.vector.tensor_tensor(out=sxsy[:], in0=sx[:], in1=sy[:], op=Alu.mult)

            ax = ep.tile([P, N_TILES], F32)
            ay = ep.tile([P, N_TILES], F32)
            nc.vector.scalar_tensor_tensor(
                out=ax[:], in0=sxsx[:], scalar=-1.0 / H, in1=sxx[:],
                op0=Alu.mult, op1=Alu.add,
            )
            nc.vector.scalar_tensor_tensor(
                out=ay[:], in0=sysy[:], scalar=-1.0 / H, in1=syy[:],
                op0=Alu.mult, op1=Alu.add,
            )
            sdx = ep.tile([P, N_TILES], F32)
            sdy = ep.tile([P, N_TILES], F32)
            nc.scalar.activation(
                out=sdx[:], in_=ax[:], func=Act.Sqrt, scale=1.0 / (H - 1)
            )
            nc.scalar.activation(
                out=sdy[:], in_=ay[:], func=Act.Sqrt, scale=1.0 / (H - 1)
            )
            nc.vector.tensor_scalar(
                out=sdx[:], in0=sdx[:], scalar1=EPSILON, scalar2=None, op0=Alu.add
            )
            nc.vector.tensor_scalar(
                out=sdy[:], in0=sdy[:], scalar1=EPSILON, scalar2=None, op0=Alu.add
            )
            den = ep.tile([P, N_TILES], F32)
            nc.vector.tensor_tensor(out=den[:], in0=sdx[:], in1=sdy[:], op=Alu.mult)
            rden = ep.tile([P, N_TILES], F32)
            nc.vector.reciprocal(out=rden[:], in_=den[:])

            num = ep.tile([P, N_TILES], F32)
            nc.vector.scalar_tensor_tensor(
                out=num[:], in0=sxsy[:], scalar=-1.0 / H, in1=sxy[:],
                op0=Alu.mult, op1=Alu.add,
            )
            corr = ep.tile([P, N_TILES], F32)
            nc.vector.scalar_tensor_tensor(
                out=corr[:], in0=num[:], scalar=1.0 / H, in1=rden[:],
                op0=Alu.mult, op1=Alu.mult,
            )

            stat3 = ep.tile([P, 4], F32)
            dead8 = ep.tile([P, N_TILES], F32)
            # col 0: sum of corr over the tile's rows
            nc.vector.tensor_scalar(
                out=dead8[:], in0=corr[:], scalar1=0.0, scalar2=None,
                op0=Alu.add, op1=Alu.add, accum_out=stat3[:, 0:1],
            )
            # col 1: sum over rows of (Sxx + Syy - 2*Sxy)
            t_m = ep.tile([P, N_TILES], F32)
            nc.vector.scalar_tensor_tensor(
                out=t_m[:], in0=sxy[:], scalar=-2.0, in1=sxx[:],
                op0=Alu.mult, op1=Alu.add,
            )
            dead8b = ep.tile([P, N_TILES], F32)
            nc.vector.scalar_tensor_tensor(
                out=dead8b[:], in0=t_m[:], scalar=0.0, in1=syy[:],
                op0=Alu.add, op1=Alu.add, accum_out=stat3[:, 1:2],
            )
            # col 2: total match count
            dead8c = ep.tile([P, N_TILES], F32)
            nc.vector.scalar_tensor_tensor(
                out=dead8c[:], in0=cnt1[:], scalar=0.0, in1=cnt2[:],
                op0=Alu.add, op1=Alu.add, accum_out=stat3[:, 2:3],
            )

            # cross-partition reduce: [1,3] = ones.T @ stat3[:, :3]
            acc = psum_pool.tile([1, 3], F32)
            nc.tensor.matmul(acc[:], ones[:], stat3[:, 0:3], start=True, stop=True)
            sb_out = ep.tile([1, 3], F32)
            nc.vector.tensor_copy(out=sb_out[:], in_=acc[:])
            nc.sync.dma_start(out=out_d[:], in_=sb_out[:])

    if split_waits:
        _split_multiwait(nc)
    return nc


_NC_CACHE = None


def _get_nc():
    global _NC_CACHE
    if _NC_CACHE is None:
        _NC_CACHE = build_bass()
    return _NC_CACHE


def run_cores(predictions, targets, **kwargs):
    """Run the SPMD kernel; returns (per-core [1,3] results list, BassKernelResults)."""
    nc = _get_nc()
    preds = np.ascontiguousarray(predictions, dtype=np.float32)
    targs = np.ascontiguousarray(targets, dtype=np.float32)
    in_maps = [
        {
            "x": preds[c * ROWS_PER_CORE : (c + 1) * ROWS_PER_CORE],
            "y": targs[c * ROWS_PER_CORE : (c + 1) * ROWS_PER_CORE],
        }
        for c in range(N_CORES)
    ]
    res = run_bass_kernel_spmd(nc, in_maps, core_ids=list(range(N_CORES)), **kwargs)
    return [r["out"] for r in res.results], res


def _combine(outs):
    corr_sum = 0.0
    mse_sum = 0.0
    cnt_sum = 0.0
    for o in outs:
        corr_sum += float(o[0, 0])
        mse_sum += float(o[0, 1])
        cnt_sum += float(o[0, 2])
    mse = mse_sum / (B_FULL * H)
    directional_loss = 1.0 - cnt_sum / (B_FULL * (H - 1))
    correlation_loss = (B_FULL - corr_sum) / (2.0 * B_FULL)
    dir_combined = (directional_loss + correlation_loss) / 2.0
    total = MSE_WEIGHT * mse + DIRECTIONAL_WEIGHT * dir_combined
    return np.float32(total)


def kernel(predictions, targets):
    outs, _ = run_cores(predictions, targets)
    return np.asarray(_combine(outs))
